# revision 1
# baseline (speedup 1.0000x reference)
"""Self-contained Trainium2 Bass kernel for the CR-VSS block (8 cores)."""

# ---- TileContext drain-wait patch (walrus 1-wait limit) ----
"""Patch TileContext._drain_and_barrier: the axon-client walrus rejects
instructions carrying >2 sem waits ("Too many sync wait commands" in
setupSyncWait for CTRL structs). Redistribute the exit-drain's waits across
preceding SP nop instructions, each carrying at most MAX_WAITS."""
from concourse.tile import TileContext, ScopedClock

MAX_WAITS = 1


def _patched_drain_and_barrier(self, tick_clock, wait_clock):
    nc = self.nc
    drain_inst = nc.sync.drain()
    wait_clock.add_sem_waits(
        drain_inst.ins, ScopedClock({None: tick_clock.global_clock})
    )

    waits = list(drain_inst.ins.sync_info.on_wait or [])
    if len(waits) > MAX_WAITS:
        bb = nc.cur_bb.bb
        assert bb.instructions[-1] is drain_inst.ins
        # strip waits from the drain, re-emit them on nop carriers
        drain_inst.ins.sync_info.on_wait = waits[:0]
        carriers = []
        import concourse.mybir as mybir
        for i in range(0, len(waits), MAX_WAITS):
            nop = nc.sync.nop(nofuse=True)
            nop.ins.sync_info = mybir.SyncInfo(
                on_wait=waits[i:i + MAX_WAITS], on_update=[]
            )
            carriers.append(nop.ins)
        # move carriers before the drain
        insts = list(bb.instructions)
        assert insts[-len(carriers) - 1] is drain_inst.ins
        reordered = insts[:-len(carriers) - 1] + insts[-len(carriers):] + [drain_inst.ins]
        while len(bb.instructions):
            bb.instructions.pop()
        for x in reordered:
            bb.instructions.append(x)

    nc.all_engine_barrier()
    assert self.sems is not None
    popped = nc._tile_sem_poison_stack.pop()
    assert popped is self._sem_poison
    nc.clear_and_free_semaphores(list(self.sems.allocated().values()))
    nc.all_engine_barrier()


def apply():
    TileContext._drain_and_barrier = _patched_drain_and_barrier


def split_multi_waits(nc, max_waits=1):
    """Post-pass: walrus CTRL codegen rejects instructions with more than
    one sem wait. Move extra waits onto same-engine NoOp carriers."""
    import concourse.mybir as mybir
    for f in nc.m.functions:
        for bb in f.blocks:
            insts = list(bb.instructions)
            out = []
            changed = False
            for ins in insts:
                si = ins.sync_info
                if si is not None and si.on_wait and len(si.on_wait) > max_waits:
                    waits = list(si.on_wait)
                    for i, w in enumerate(waits[max_waits:]):
                        nop = mybir.InstNoOp.__new__(
                            mybir.InstNoOp, name=f"{ins.name}-xw{i}", ins=[], outs=[])
                        nop.engine = ins.engine
                        nop.sync_info = mybir.SyncInfo(on_wait=[w], on_update=[])
                        out.append(nop)
                    ins.sync_info = mybir.SyncInfo(
                        on_wait=waits[:max_waits],
                        on_update=list(si.on_update or []))
                    changed = True
                out.append(ins)
            if changed:
                while len(bb.instructions):
                    bb.instructions.pop()
                for x in out:
                    bb.instructions.append(x)

apply()

# ---- kernel ----
"""Trainium2 Bass kernel for nn_CR_VSS (VSS block with SS2D selective scan).

Sharding: 8 cores = 4 samples x 2 d_inner-halves. Each core runs the full
pre-stage for its sample (duplicated within the pair), scans its 96-channel
d-half across all 4 cross-scan directions (packed into 3x128-partition
tiles), and the pair exchanges LayerNorm stats + out-proj partial sums via
2-core AllReduce. Post-stage is computed redundantly per pair.

Scan: h_t = exp(A*delta_t)*h_{t-1} + delta_t*u_t*B_t per (k,d,n) via the
DVE tensor_tensor_scan; n as an outer loop; y accumulated over n with
identity-lhsT PSUM matmuls.
"""
import numpy as np
from contextlib import ExitStack

import concourse.bass as bass
import concourse.mybir as mybir
from concourse.tile import TileContext

F = mybir.ActivationFunctionType
A = mybir.AluOpType
FP32 = mybir.dt.float32
BF16 = mybir.dt.bfloat16

B_, CIN, CH, COUT, H, W = 4, 96, 96, 96, 48, 48
DI, N, R, K4 = 192, 16, 6, 4
L = H * W               # 2304
HH = 96                 # d-half per core
NT = 3                  # packed (k,d) tiles: 4*96 = 384 = 3*128
HP = 50
LP = 2500
TC = 768                # scan t-chunk (16 rows of 48)
TCH = [(0, 768), (768, 1536), (1536, 2304)]

# packed row r = k*96 + d -> (tile j, offset): sections (j, o0, o1, k, d0, d1)
SECTIONS = [
    (0, 0, 96, 0, 0, 96),
    (0, 96, 128, 1, 0, 32),
    (1, 0, 32, 1, 32, 64),
    (1, 32, 64, 1, 64, 96),
    (1, 64, 128, 2, 0, 64),
    (2, 0, 32, 2, 64, 96),
    (2, 32, 64, 3, 0, 32),
    (2, 64, 96, 3, 32, 64),
    (2, 96, 128, 3, 64, 96),
]

MM_CHUNKS = [(0, 512), (512, 1024), (1024, 1536), (1536, 2048), (2048, 2304)]
ROW_CHUNKS = [(0, 10), (10, 20), (20, 30), (30, 40), (40, 48)]
SUBS768 = [(0, 512), (512, 768)]

REPLICA_GROUPS = [[0, 1], [2, 3], [4, 5], [6, 7]]


def build_nc():
    nc = bass.Bass(trn_type="TRN2", num_devices=8)

    def din(name, shape, d=FP32):
        return nc.dram_tensor(name, list(shape), d, kind="ExternalInput")

    x_d = din("x", (CIN, L))
    w1T_d = din("w1T", (CIN, CH))
    b1_d = din("b1", (CH, 1))
    linT_d = din("linT", (CH, CH))
    linb_d = din("linb", (CH, 1))
    dw1dg_d = din("dw1dg", (CH, 9 * CH))
    dw1b_d = din("dw1b", (CH, 1))
    dw2dg_d = din("dw2dg", (CH, 9 * CH))
    dw2b_d = din("dw2b", (CH, 1))
    inwT_d = din("inwT", (CH, DI + HH), BF16)
    sc0dg_d = din("sc0dg", (128, 9 * 128), BF16)
    sc1dg_d = din("sc1dg", (64, 9 * 64), BF16)
    scb0_d = din("scb0", (128, 1))
    scb1_d = din("scb1", (64, 1))
    sel0_d = din("sel0", (128, HH), BF16)
    sel1_d = din("sel1", (64, HH), BF16)
    xpTa_d = din("xpTa", (128, K4 * 96), BF16)
    xpTb_d = din("xpTb", (64, K4 * 96), BF16)
    dtwT_d = din("dtwT", (R, K4 * HH), BF16)
    dtb_d = din("dtb", (128, NT))
    Ap_d = din("Ap", (128, NT * N))
    Dp_d = din("Dp", (128, NT))
    ident_d = din("ident", (128, 128), BF16)
    outng_d = din("outng", (HH, 1))
    outnb_d = din("outnb", (HH, 1))
    outwT_d = din("outwT", (HH, CH), BF16)
    ag1T_d = din("ag1T", (CH, 48), BF16)
    ag1b_d = din("ag1b", (48, 1))
    ag2T_d = din("ag2T", (48, CH), BF16)
    ag2b_d = din("ag2b", (CH, 1))
    lng_d = din("lng", (CH, 1))
    lnb_d = din("lnb", (CH, 1))
    sq1T_d = din("sq1T", (48, 24), BF16)
    sq2T_d = din("sq2T", (48, 24), BF16)
    gwcT_d = din("gwcT", (24, 9 * CH), BF16)
    gwcb_d = din("gwcb", (CH, 1))
    pw1T_d = din("pw1T", (24, CH), BF16)
    pw2T_d = din("pw2T", (24, 72), BF16)
    finT_d = din("finT", (CH, COUT), BF16)
    finb_d = din("finb", (COUT, 1))

    out_d = nc.dram_tensor("out", [COUT, L], FP32, kind="ExternalOutput")

    B_dram = nc.dram_tensor("B_dram", [K4 * N, L], BF16)
    C_dram = nc.dram_tensor("C_dram", [K4 * N, L], BF16)
    st1_dram = nc.dram_tensor("st1_dram", [2, L], FP32)
    st1r_dram = nc.dram_tensor("st1r_dram", [2, L], FP32)
    x1o_dram = nc.dram_tensor("x1o_dram", [CH, L], FP32)
    x1or_dram = nc.dram_tensor("x1or_dram", [CH, L], FP32)

    def hw(ap):
        return ap.rearrange("p (h w) -> p h w", h=H)

    def hwp(ap):
        return ap.rearrange("p (h w) -> p h w", h=HP)

    def whv(ap):
        return ap.rearrange("p (h w) -> p w h", h=H)

    with TileContext(nc) as tc:
        glob = ExitStack()
        cst = glob.enter_context(tc.tile_pool(name="cst", bufs=1))
        lng_p = glob.enter_context(tc.tile_pool(name="lng_p", bufs=1))

        def cload(d, shape, dty=FP32, name=None):
            t = cst.tile(list(shape), dty, tag=name)
            nc.sync.dma_start(t[:], d[:])
            return t

        w1T = cload(w1T_d, (CIN, CH), name="w1T")
        b1 = cload(b1_d, (CH, 1), name="b1")
        linT = cload(linT_d, (CH, CH), name="linT")
        linb = cload(linb_d, (CH, 1), name="linb")
        dw1dg = cload(dw1dg_d, (CH, 9 * CH), name="dw1dg")
        dw1b = cload(dw1b_d, (CH, 1), name="dw1b")
        dw2dg = cload(dw2dg_d, (CH, 9 * CH), name="dw2dg")
        dw2b = cload(dw2b_d, (CH, 1), name="dw2b")
        inwT = cload(inwT_d, (CH, DI + HH), BF16, name="inwT")
        sc0dg = cload(sc0dg_d, (128, 9 * 128), BF16, name="sc0dg")
        sc1dg = cload(sc1dg_d, (64, 9 * 64), BF16, name="sc1dg")
        scb0 = cload(scb0_d, (128, 1), name="scb0")
        scb1 = cload(scb1_d, (64, 1), name="scb1")
        sel0 = cload(sel0_d, (128, HH), BF16, name="sel0")
        sel1 = cload(sel1_d, (64, HH), BF16, name="sel1")
        xpTa = cload(xpTa_d, (128, K4 * 96), BF16, name="xpTa")
        xpTb = cload(xpTb_d, (64, K4 * 96), BF16, name="xpTb")
        dtwT = cload(dtwT_d, (R, K4 * HH), BF16, name="dtwT")
        dtb = cload(dtb_d, (128, NT), name="dtb")
        Apt = cload(Ap_d, (128, NT * N), name="Apt")
        Dpt = cload(Dp_d, (128, NT), name="Dpt")
        ident = cload(ident_d, (128, 128), BF16, name="ident")
        outng = cload(outng_d, (HH, 1), name="outng")
        outnb = cload(outnb_d, (HH, 1), name="outnb")
        outwT = cload(outwT_d, (HH, CH), BF16, name="outwT")
        ag1T = cload(ag1T_d, (CH, 48), BF16, name="ag1T")
        ag1b = cload(ag1b_d, (48, 1), name="ag1b")
        ag2T = cload(ag2T_d, (48, CH), BF16, name="ag2T")
        ag2b = cload(ag2b_d, (CH, 1), name="ag2b")
        lng = cload(lng_d, (CH, 1), name="lng")
        lnb = cload(lnb_d, (CH, 1), name="lnb")
        sq1T = cload(sq1T_d, (48, 24), BF16, name="sq1T")
        sq2T = cload(sq2T_d, (48, 24), BF16, name="sq2T")
        gwcT = cload(gwcT_d, (24, 9 * CH), BF16, name="gwcT")
        gwcb = cload(gwcb_d, (CH, 1), name="gwcb")
        pw1T = cload(pw1T_d, (24, CH), BF16, name="pw1T")
        pw2T = cload(pw2T_d, (24, 72), BF16, name="pw2T")
        finT = cload(finT_d, (CH, COUT), BF16, name="finT")
        finb = cload(finb_d, (COUT, 1), name="finb")

        ones96 = cst.tile([HH, 1], FP32, tag="ones96")
        nc.vector.memset(ones96[:], 1.0)

        # long-lived across phases
        zt = lng_p.tile([HH, L], BF16, tag="zt")
        x2 = lng_p.tile([CH, L], BF16, tag="x2")
        mid = ExitStack()
        mid_p = mid.enter_context(tc.tile_pool(name="mid_p", bufs=1))
        dp = [mid_p.tile([128, L], BF16, tag=f"dp{j}", name=f"dp{j}") for j in range(NT)]
        dlu = [mid_p.tile([128, L], BF16, tag=f"dlu{j}", name=f"dlu{j}") for j in range(NT)]
        xsp = [mid_p.tile([128, L], BF16, tag=f"xsp{j}", name=f"xsp{j}") for j in range(NT)]


        # ================= pre-stage =================
        pre = ExitStack()
        pre_ps = pre.enter_context(tc.tile_pool(name="pre_ps", bufs=4, space="PSUM"))
        pA = pre.enter_context(tc.tile_pool(name="pA", bufs=1))
        pB = pre.enter_context(tc.tile_pool(name="pB", bufs=1))

        xt = pA.tile([CIN, L], FP32, tag="af", bufs=2, name="xt")
        nc.sync.dma_start(xt[:], x_d[:])
        h1 = pA.tile([CH, L], FP32, tag="af", bufs=2, name="h1")
        for c0, c1 in MM_CHUNKS:
            ps = pre_ps.tile([CH, 512], FP32, tag="ps")
            nc.tensor.matmul(ps[:, :c1 - c0], w1T[:], xt[:, c0:c1], start=True, stop=True)
            nc.scalar.activation(h1[:, c0:c1], ps[:, :c1 - c0], F.Relu, bias=b1[:])
        h2 = pA.tile([CH, L], FP32, tag="af", bufs=2, name="h2")
        for c0, c1 in MM_CHUNKS:
            ps = pre_ps.tile([CH, 512], FP32, tag="ps")
            nc.tensor.matmul(ps[:, :c1 - c0], linT[:], h1[:, c0:c1], start=True, stop=True)
            nc.scalar.activation(h2[:, c0:c1], ps[:, :c1 - c0], F.Identity, bias=linb[:])
        h2p = pA.tile([CH, LP], FP32, tag="h2p")
        nc.vector.memset(h2p[:], 0.0)
        nc.vector.tensor_copy(hwp(h2p[:])[:, 1:49, 1:49], hw(h2[:]))

        def dwconv(dst, src_p, diag, bias, nch):
            for (r0, r1) in ROW_CHUNKS:
                nr = r1 - r0
                ps = pre_ps.tile([128, 480], FP32, tag="ps")
                for tap in range(9):
                    dy, dx = tap // 3, tap % 3
                    rhs = hwp(src_p[:])[:, dy + r0:dy + r1, dx:dx + 48]
                    nc.tensor.matmul(ps[:nch, :nr * 48],
                                     diag[:, tap * nch:(tap + 1) * nch],
                                     rhs, start=(tap == 0), stop=(tap == 8))
                # silu = x * sigmoid(x) (sim has no native Silu)
                xa = pB.tile([128, 480], FP32, tag="dwtmp", bufs=3, name="xa")
                nc.scalar.activation(xa[:nch, :nr * 48], ps[:nch, :nr * 48],
                                     F.Identity, bias=bias[:])
                sg = pre_ps.tile([128, 480], FP32, tag="ps")
                nc.scalar.activation(sg[:nch, :nr * 48], ps[:nch, :nr * 48],
                                     F.Sigmoid, bias=bias[:])
                nc.vector.tensor_mul(dst[:, r0 * 48:r1 * 48], xa[:nch, :nr * 48],
                                     sg[:nch, :nr * 48])

        x1 = pB.tile([CH, L], BF16, tag="x1")
        dwconv(x1, h2p, dw1dg, dw1b, CH)
        dwconv(x2, h2p, dw2dg, dw2b, CH)

        xi0 = pB.tile([128, L], BF16, tag="xi0")
        xi1 = pB.tile([64, L], BF16, tag="xi1")
        for mb, (m0, m1) in enumerate([(0, 128), (128, 256), (256, 288)]):
            for c0, c1 in MM_CHUNKS:
                ps = pre_ps.tile([128, 512], FP32, tag="ps")
                nc.tensor.matmul(ps[:m1 - m0, :c1 - c0], inwT[:, m0:m1],
                                 x1[:, c0:c1], start=True, stop=True)
                if mb == 0:
                    nc.scalar.copy(xi0[:, c0:c1], ps[:128, :c1 - c0])
                elif mb == 1:
                    nc.scalar.copy(xi1[:, c0:c1], ps[0:64, :c1 - c0])
                    nc.scalar.copy(zt[0:64, c0:c1], ps[64:128, :c1 - c0])
                else:
                    nc.scalar.copy(zt[64:96, c0:c1], ps[0:32, :c1 - c0])

        xi0p = pB.tile([128, LP], BF16, tag="xi0p")
        xi1p = pB.tile([64, LP], BF16, tag="xi1p")
        nc.vector.memset(xi0p[:], 0.0)
        nc.vector.memset(xi1p[:], 0.0)
        nc.vector.tensor_copy(hwp(xi0p[:])[:, 1:49, 1:49], hw(xi0[:]))
        nc.vector.tensor_copy(hwp(xi1p[:])[:, 1:49, 1:49], hw(xi1[:]))
        xc0 = pB.tile([128, L], BF16, tag="xc0")
        xc1 = pB.tile([64, L], BF16, tag="xc1")
        dwconv(xc0, xi0p, sc0dg, scb0, 128)
        dwconv(xc1, xi1p, sc1dg, scb1, 64)

        # xproj (row-chunked so wh views stay rectangular)
        dts = [pB.tile([R, L], BF16, tag=f"dts{k}", name=f"dts{k}") for k in range(K4)]

        def xc_read(k, c0, c1):
            """Full-d xc in direction-k scan order, flat chunk (48-aligned)."""
            if k == 0:
                return (xc0[:, c0:c1], xc1[:, c0:c1])
            if k == 1:
                return (whv(xc0[:])[:, c0 // 48:c1 // 48, :],
                        whv(xc1[:])[:, c0 // 48:c1 // 48, :])
            if k == 2:
                return (xc0[:, L - c1:L - c0][:, ::-1],
                        xc1[:, L - c1:L - c0][:, ::-1])
            r0 = whv(xc0[:])[:, (L - c1) // 48:(L - c0) // 48, :][:, ::-1, ::-1]
            r1 = whv(xc1[:])[:, (L - c1) // 48:(L - c0) // 48, :][:, ::-1, ::-1]
            return (r0, r1)

        for k in range(K4):
            for (rr0, rr1) in ROW_CHUNKS:
                c0, c1 = rr0 * 48, rr1 * 48
                nf = c1 - c0
                ra, rb = xc_read(k, c0, c1)
                ps = pre_ps.tile([96, 480], FP32, tag="ps")
                nc.tensor.matmul(ps[:, :nf], xpTa[:, k * 96:(k + 1) * 96], ra,
                                 start=True, stop=False)
                nc.tensor.matmul(ps[:, :nf], xpTb[:, k * 96:(k + 1) * 96], rb,
                                 start=False, stop=True)
                nc.scalar.copy(dts[k][:, c0:c1], ps[0:R, :nf])
                bstg = pB.tile([N, 480], BF16, tag="bstg", bufs=3, name="bstg")
                cstg = pB.tile([N, 480], BF16, tag="cstg", bufs=3, name="cstg")
                nc.scalar.copy(bstg[:, :nf], ps[32:32 + N, :nf])
                nc.scalar.copy(cstg[:, :nf], ps[64:64 + N, :nf])
                nc.sync.dma_start(B_dram[k * N:(k + 1) * N, c0:c1], bstg[:, :nf])
                nc.sync.dma_start(C_dram[k * N:(k + 1) * N, c0:c1], cstg[:, :nf])

        # delta (softplus) into packed tiles
        for k in range(K4):
            for c0, c1 in MM_CHUNKS:
                ps = pre_ps.tile([HH, 512], FP32, tag="ps")
                nc.tensor.matmul(ps[:, :c1 - c0], dtwT[:, k * HH:(k + 1) * HH],
                                 dts[k][:, c0:c1], start=True, stop=True)
                for (j, o0, o1, kk, d0, d1) in SECTIONS:
                    if kk != k:
                        continue
                    # softplus(x+b) = ln(1 + exp(x+b)) (sim has no Softplus)
                    ex = pre_ps.tile([128, 512], FP32, tag="ps")
                    nc.scalar.activation(
                        ex[o0:o1, :c1 - c0], ps[d0:d1, :c1 - c0],
                        F.Exp, bias=dtb[o0:o1, j:j + 1])
                    nc.scalar.activation(
                        dp[j][o0:o1, c0:c1], ex[o0:o1, :c1 - c0],
                        F.Ln, bias=1.0)

        # xs half extraction + wh copy
        xch = pB.tile([HH, L], BF16, tag="xch")
        xwhh = pB.tile([HH, L], BF16, tag="xwhh")
        for c0, c1 in MM_CHUNKS:
            ps = pre_ps.tile([HH, 512], FP32, tag="ps")
            nc.tensor.matmul(ps[:, :c1 - c0], sel0[:], xc0[:, c0:c1], start=True, stop=False)
            nc.tensor.matmul(ps[:, :c1 - c0], sel1[:], xc1[:, c0:c1], start=False, stop=True)
            nc.scalar.copy(xch[:, c0:c1], ps[:, :c1 - c0])
        nc.vector.tensor_copy(xwhh[:], whv(xch[:]))

        # pack xs (scan order) into the (k,d) tile layout via DMA repack
        for (j, o0, o1, k, d0, d1) in SECTIONS:
            if k == 0:
                srcap = xch[d0:d1, :]
            elif k == 1:
                srcap = xwhh[d0:d1, :]
            elif k == 2:
                srcap = xch[d0:d1, ::-1]
            else:
                srcap = xwhh[d0:d1, ::-1]
            nc.sync.dma_start(xsp[j][o0:o1, :], srcap)
        for j in range(NT):
            nc.vector.tensor_mul(dlu[j][:], dp[j][:], xsp[j][:])

        pre.close()

        # ================= scan =================
        sc = ExitStack()
        so = ExitStack()
        so_p = so.enter_context(tc.tile_pool(name="so_p", bufs=1))
        ydp = [so_p.tile([128, L], BF16, tag=f"ydp{j}", name=f"ydp{j}") for j in range(NT)]
        yd = [so_p.tile([HH, L], BF16, tag=f"yd{k}", name=f"yd{k}") for k in range(K4)]
        scan_ps = sc.enter_context(tc.tile_pool(name="scan_ps", bufs=1, space="PSUM"))
        spool = sc.enter_context(tc.tile_pool(name="spool", bufs=2))
        ypsum = [scan_ps.tile([128, TC], FP32, tag=f"yps{j}", name=f"yps{j}") for j in range(NT)]
        state = [sc.enter_context(tc.tile_pool(name=f"st{j}", bufs=1)).tile(
            [128, N], FP32, tag=f"state{j}", name=f"state{j}") for j in range(NT)]

        for ci, (c0, c1) in enumerate(TCH):
            for n in range(N):
                for j in range(NT):
                    at = spool.tile([128, TC], FP32, tag=f"a{j}", name=f"a{j}")
                    nc.scalar.activation(at[:], dp[j][:, c0:c1], F.Exp,
                                         scale=Apt[:, j * N + n:j * N + n + 1])
                    Bb = spool.tile([128, TC], BF16, tag=f"Bb{j}", name=f"Bb{j}")
                    Cb = spool.tile([128, TC], BF16, tag=f"Cb{j}", name=f"Cb{j}")
                    for (jj, o0, o1, k, d0, d1) in SECTIONS:
                        if jj != j:
                            continue
                        nc.sync.dma_start(
                            Bb[o0:o1, :],
                            B_dram[k * N + n, c0:c1].partition_broadcast(o1 - o0))
                        nc.sync.dma_start(
                            Cb[o0:o1, :],
                            C_dram[k * N + n, c0:c1].partition_broadcast(o1 - o0))
                    bt = spool.tile([128, TC], BF16, tag=f"b{j}", name=f"b{j}")
                    nc.vector.tensor_mul(bt[:], dlu[j][:, c0:c1], Bb[:])
                    ht = spool.tile([128, TC], BF16, tag=f"h{j}", name=f"h{j}")
                    init = 0.0 if ci == 0 else state[j][:, n:n + 1]
                    nc.vector.tensor_tensor_scan(ht[:], at[:], bt[:], init,
                                                 A.mult, A.add)
                    if ci < 2:
                        nc.vector.tensor_copy(state[j][:, n:n + 1], ht[:, TC - 1:TC])
                    gt = spool.tile([128, TC], BF16, tag=f"g{j}", name=f"g{j}")
                    nc.vector.tensor_mul(gt[:], ht[:], Cb[:])
                    for (s0, s1) in SUBS768:
                        nc.tensor.matmul(ypsum[j][:, s0:s1], ident[:], gt[:, s0:s1],
                                         start=(n == 0), stop=(n == N - 1))
            for j in range(NT):
                nc.vector.scalar_tensor_tensor(
                    out=ydp[j][:, c0:c1], in0=xsp[j][:, c0:c1],
                    scalar=Dpt[:, j:j + 1],
                    in1=ypsum[j][:], op0=A.mult, op1=A.add)
        sc.close()
        for (j, o0, o1, k, d0, d1) in SECTIONS:
            nc.sync.dma_start(yd[k][d0:d1, :], ydp[j][o0:o1, :])

        # ================= merge + out-norm =================
        ysum = lng_p.tile([HH, L], FP32, tag="ysum")
        tmp2 = lng_p.tile([HH, L], FP32, tag="tmp2")
        nc.vector.tensor_add(ysum[:], yd[0][:], yd[2][:, ::-1])
        nc.vector.tensor_add(tmp2[:], yd[1][:], yd[3][:, ::-1])
        nc.vector.tensor_add(ysum[:], ysum[:], whv(tmp2[:]))
        so.close()
        mid.close()

        po = ExitStack()
        post_ps = po.enter_context(tc.tile_pool(name="post_ps", bufs=4, space="PSUM"))
        pP = po.enter_context(tc.tile_pool(name="pP", bufs=1))
        rot = po.enter_context(tc.tile_pool(name="rot", bufs=4))

        ysq = rot.tile([HH, L], FP32, tag="pf")
        nc.scalar.activation(ysq[:], ysum[:], F.Square)
        s1 = pP.tile([1, L], FP32, tag="stx", bufs=2, name="s1")
        s2 = pP.tile([1, L], FP32, tag="stx", bufs=2, name="s2")
        for c0, c1 in MM_CHUNKS:
            ps = post_ps.tile([1, 512], FP32, tag="ps")
            nc.tensor.matmul(ps[:, :c1 - c0], ones96[:], ysum[:, c0:c1], start=True, stop=True)
            nc.scalar.copy(s1[:, c0:c1], ps[:, :c1 - c0])
            ps2 = post_ps.tile([1, 512], FP32, tag="ps")
            nc.tensor.matmul(ps2[:, :c1 - c0], ones96[:], ysq[:, c0:c1], start=True, stop=True)
            nc.scalar.copy(s2[:, c0:c1], ps2[:, :c1 - c0])
        nc.sync.dma_start(st1_dram[0:1, :], s1[:])
        nc.sync.dma_start(st1_dram[1:2, :], s2[:])
        nc.gpsimd.collective_compute(
            "AllReduce", A.add, replica_groups=REPLICA_GROUPS,
            ins=[st1_dram[:]], outs=[st1r_dram[:]])
        mean_b = rot.tile([HH, L], FP32, tag="pf")
        m2_b = rot.tile([HH, L], FP32, tag="pf")
        nc.sync.dma_start(mean_b[:], st1r_dram[0, :].partition_broadcast(HH))
        nc.sync.dma_start(m2_b[:], st1r_dram[1, :].partition_broadcast(HH))
        cDI = 1.0 / DI
        nc.vector.tensor_scalar(out=mean_b[:], in0=mean_b[:], scalar1=cDI,
                                scalar2=None, op0=A.mult)
        msq = rot.tile([HH, L], FP32, tag="pf")
        nc.scalar.activation(msq[:], mean_b[:], F.Square)
        var = rot.tile([HH, L], FP32, tag="pf")
        nc.vector.scalar_tensor_tensor(out=var[:], in0=m2_b[:], scalar=cDI,
                                       in1=msq[:], op0=A.mult, op1=A.subtract)
        nc.vector.tensor_scalar(out=var[:], in0=var[:], scalar1=1e-5,
                                scalar2=None, op0=A.add)
        rstd = rot.tile([HH, L], FP32, tag="pf")
        nc.vector.reciprocal(rstd[:], var[:])
        nc.scalar.activation(rstd[:], rstd[:], F.Sqrt)
        yn = rot.tile([HH, L], FP32, tag="pf")
        nc.vector.tensor_sub(yn[:], ysum[:], mean_b[:])
        nc.vector.tensor_mul(yn[:], yn[:], rstd[:])
        nc.vector.tensor_scalar(out=yn[:], in0=yn[:], scalar1=outng[:, 0:1],
                                scalar2=outnb[:, 0:1], op0=A.mult, op1=A.add)
        zs = rot.tile([HH, L], BF16, tag="pb")
        nc.scalar.activation(zs[:], zt[:], F.Sigmoid)
        nc.vector.tensor_mul(zs[:], zs[:], zt[:])
        ygz = rot.tile([HH, L], BF16, tag="pb")
        nc.vector.tensor_mul(ygz[:], yn[:], zs[:])

        x1o_p = pP.tile([CH, L], FP32, tag="x1o_p")
        for c0, c1 in MM_CHUNKS:
            ps = post_ps.tile([CH, 512], FP32, tag="ps")
            nc.tensor.matmul(ps[:, :c1 - c0], outwT[:], ygz[:, c0:c1], start=True, stop=True)
            nc.scalar.copy(x1o_p[:, c0:c1], ps[:, :c1 - c0])
        nc.sync.dma_start(x1o_dram[:], x1o_p[:])
        nc.gpsimd.collective_compute(
            "AllReduce", A.add, replica_groups=REPLICA_GROUPS,
            ins=[x1o_dram[:]], outs=[x1or_dram[:]])
        x1o = pP.tile([CH, L], FP32, tag="x1o")
        nc.sync.dma_start(x1o[:], x1or_dram[:])

        # branch 2 (attention gate)
        g1 = rot.tile([48, L], BF16, tag="pb")
        for c0, c1 in MM_CHUNKS:
            ps = post_ps.tile([48, 512], FP32, tag="ps")
            nc.tensor.matmul(ps[:, :c1 - c0], ag1T[:], x2[:, c0:c1], start=True, stop=True)
            nc.scalar.activation(g1[:, c0:c1], ps[:, :c1 - c0], F.Relu, bias=ag1b[:])
        gat = rot.tile([CH, L], BF16, tag="pb")
        for c0, c1 in MM_CHUNKS:
            ps = post_ps.tile([CH, 512], FP32, tag="ps")
            nc.tensor.matmul(ps[:, :c1 - c0], ag2T[:], g1[:, c0:c1], start=True, stop=True)
            nc.scalar.activation(gat[:, c0:c1], ps[:, :c1 - c0], F.Sigmoid, bias=ag2b[:])
        x2g = rot.tile([CH, L], BF16, tag="pb")
        nc.vector.tensor_mul(x2g[:], x2[:], gat[:])
        yb = pP.tile([CH, L], FP32, tag="yb")
        nc.vector.tensor_add(yb[:], x1o[:], x2g[:])

        # local LayerNorm over channels
        ybsq = rot.tile([CH, L], FP32, tag="pf")
        nc.scalar.activation(ybsq[:], yb[:], F.Square)
        t1 = pP.tile([1, L], FP32, tag="stx", bufs=2, name="t1")
        t2 = pP.tile([1, L], FP32, tag="stx", bufs=2, name="t2")
        for c0, c1 in MM_CHUNKS:
            ps = post_ps.tile([1, 512], FP32, tag="ps")
            nc.tensor.matmul(ps[:, :c1 - c0], ones96[:], yb[:, c0:c1], start=True, stop=True)
            nc.scalar.copy(t1[:, c0:c1], ps[:, :c1 - c0])
            ps2 = post_ps.tile([1, 512], FP32, tag="ps")
            nc.tensor.matmul(ps2[:, :c1 - c0], ones96[:], ybsq[:, c0:c1], start=True, stop=True)
            nc.scalar.copy(t2[:, c0:c1], ps2[:, :c1 - c0])
        nc.sync.dma_start(st1_dram[0:1, :], t1[:])
        nc.sync.dma_start(st1_dram[1:2, :], t2[:])
        mean2 = rot.tile([CH, L], FP32, tag="pf")
        m22 = rot.tile([CH, L], FP32, tag="pf")
        nc.sync.dma_start(mean2[:], st1_dram[0, :].partition_broadcast(CH))
        nc.sync.dma_start(m22[:], st1_dram[1, :].partition_broadcast(CH))
        cCH = 1.0 / CH
        nc.vector.tensor_scalar(out=mean2[:], in0=mean2[:], scalar1=cCH,
                                scalar2=None, op0=A.mult)
        msq2 = rot.tile([CH, L], FP32, tag="pf")
        nc.scalar.activation(msq2[:], mean2[:], F.Square)
        var2 = rot.tile([CH, L], FP32, tag="pf")
        nc.vector.scalar_tensor_tensor(out=var2[:], in0=m22[:], scalar=cCH,
                                       in1=msq2[:], op0=A.mult, op1=A.subtract)
        nc.vector.tensor_scalar(out=var2[:], in0=var2[:], scalar1=1e-5,
                                scalar2=None, op0=A.add)
        rstd2 = rot.tile([CH, L], FP32, tag="pf")
        nc.vector.reciprocal(rstd2[:], var2[:])
        nc.scalar.activation(rstd2[:], rstd2[:], F.Sqrt)
        ybn = pP.tile([CH, L], BF16, tag="ybn")
        nc.vector.tensor_sub(ybn[:], yb[:], mean2[:])
        nc.vector.tensor_mul(ybn[:], ybn[:], rstd2[:])
        nc.vector.tensor_scalar(out=ybn[:], in0=ybn[:], scalar1=lng[:, 0:1],
                                scalar2=lnb[:, 0:1], op0=A.mult, op1=A.add)

        # CRM
        low_t = rot.tile([48, L], BF16, tag="pb")
        nc.sync.dma_start(low_t[:], ybn[48:96, :])
        upc = pP.tile([24, L], BF16, tag="upc")
        lowc = pP.tile([24, L], BF16, tag="lowc")
        for c0, c1 in MM_CHUNKS:
            ps = post_ps.tile([24, 512], FP32, tag="ps")
            nc.tensor.matmul(ps[:, :c1 - c0], sq1T[:], ybn[0:48, c0:c1], start=True, stop=True)
            nc.scalar.copy(upc[:, c0:c1], ps[:, :c1 - c0])
            ps2 = post_ps.tile([24, 512], FP32, tag="ps")
            nc.tensor.matmul(ps2[:, :c1 - c0], sq2T[:], low_t[:, c0:c1], start=True, stop=True)
            nc.scalar.copy(lowc[:, c0:c1], ps2[:, :c1 - c0])
        upcp = pP.tile([24, LP], BF16, tag="upcp")
        nc.vector.memset(upcp[:], 0.0)
        nc.vector.tensor_copy(hwp(upcp[:])[:, 1:49, 1:49], hw(upc[:]))
        Y1 = pP.tile([CH, L], BF16, tag="Y1")
        m1c = pP.tile([CH, 5], FP32, tag="m1c")
        for ri, (r0, r1) in enumerate(ROW_CHUNKS):
            nr = r1 - r0
            ps = post_ps.tile([CH, 480], FP32, tag="ps")
            for tap in range(9):
                dy, dx = tap // 3, tap % 3
                rhs = hwp(upcp[:])[:, dy + r0:dy + r1, dx:dx + 48]
                nc.tensor.matmul(ps[:, :nr * 48], gwcT[:, tap * CH:(tap + 1) * CH],
                                 rhs, start=(tap == 0), stop=False)
            nc.tensor.matmul(ps[:, :nr * 48], pw1T[:], upc[:, r0 * 48:r1 * 48],
                             start=False, stop=True)
            nc.scalar.activation(Y1[:, r0 * 48:r1 * 48], ps[:, :nr * 48],
                                 F.Identity, bias=gwcb[:],
                                 accum_out=m1c[:, ri:ri + 1])
        Y2a = pP.tile([72, L], BF16, tag="Y2a")
        m2ca = pP.tile([72, 5], FP32, tag="m2ca")
        m2cb = pP.tile([24, 5], FP32, tag="m2cb")
        for ri, (c0, c1) in enumerate(MM_CHUNKS):
            ps = post_ps.tile([72, 512], FP32, tag="ps")
            nc.tensor.matmul(ps[:, :c1 - c0], pw2T[:], lowc[:, c0:c1], start=True, stop=True)
            nc.scalar.activation(Y2a[:, c0:c1], ps[:, :c1 - c0], F.Identity,
                                 accum_out=m2ca[:, ri:ri + 1])
            scr = post_ps.tile([24, 512], FP32, tag="ps")
            nc.scalar.activation(scr[:, :c1 - c0], lowc[:, c0:c1], F.Identity,
                                 accum_out=m2cb[:, ri:ri + 1])
        m1 = pP.tile([CH, 1], FP32, tag="m1")
        m2a_s = pP.tile([72, 1], FP32, tag="m2a_s")
        m2b_s = pP.tile([24, 1], FP32, tag="m2b_s")
        nc.vector.reduce_sum(m1[:], m1c[:], axis=mybir.AxisListType.X)
        nc.vector.reduce_sum(m2a_s[:], m2ca[:], axis=mybir.AxisListType.X)
        nc.vector.reduce_sum(m2b_s[:], m2cb[:], axis=mybir.AxisListType.X)
        smf = pP.tile([1, 2 * CH], FP32, tag="smf")
        nc.sync.dma_start(smf[0:1, 0:CH], m1[:, 0:1])
        nc.sync.dma_start(smf[0:1, CH:CH + 72], m2a_s[:, 0:1])
        nc.sync.dma_start(smf[0:1, CH + 72:2 * CH], m2b_s[:, 0:1])
        nc.vector.tensor_scalar(out=smf[:], in0=smf[:], scalar1=1.0 / L,
                                scalar2=None, op0=A.mult)
        mx = pP.tile([1, 1], FP32, tag="mx")
        nc.vector.reduce_max(mx[:], smf[:], axis=mybir.AxisListType.X)
        nc.vector.tensor_scalar(out=mx[:], in0=mx[:], scalar1=-1.0,
                                scalar2=None, op0=A.mult)
        nc.scalar.activation(smf[:], smf[:], F.Exp, bias=mx[0:1, 0:1])
        sm_s = pP.tile([1, 1], FP32, tag="sm_s")
        nc.vector.reduce_sum(sm_s[:], smf[:], axis=mybir.AxisListType.X)
        nc.vector.reciprocal(sm_s[:], sm_s[:])
        nc.vector.tensor_scalar(out=smf[:], in0=smf[:], scalar1=sm_s[0:1, 0:1],
                                scalar2=None, op0=A.mult)
        sm1 = pP.tile([CH, 1], FP32, tag="sm1")
        sm2 = pP.tile([CH, 1], FP32, tag="sm2")
        nc.sync.dma_start(sm1[:, 0:1], smf[0:1, 0:CH])
        nc.sync.dma_start(sm2[:, 0:1], smf[0:1, CH:2 * CH])
        o2f = rot.tile([CH, L], BF16, tag="pb")
        nc.sync.dma_start(o2f[0:72, :], Y2a[:])
        nc.sync.dma_start(o2f[72:96, :], lowc[:])
        o2t = rot.tile([CH, L], FP32, tag="pf")
        nc.vector.tensor_scalar(out=o2t[:], in0=o2f[:], scalar1=sm2[:, 0:1],
                                scalar2=None, op0=A.mult)
        yc = pP.tile([CH, L], BF16, tag="yc")
        nc.vector.scalar_tensor_tensor(out=yc[:], in0=Y1[:], scalar=sm1[:, 0:1],
                                       in1=o2t[:], op0=A.mult, op1=A.add)
        outt = pP.tile([COUT, L], FP32, tag="outt")
        for c0, c1 in MM_CHUNKS:
            ps = post_ps.tile([COUT, 512], FP32, tag="ps")
            nc.tensor.matmul(ps[:, :c1 - c0], finT[:], yc[:, c0:c1], start=True, stop=True)
            nc.scalar.activation(outt[:, c0:c1], ps[:, :c1 - c0], F.Identity, bias=finb[:])
        nc.sync.dma_start(out_d[:], outt[:])
        po.close()
        glob.close()
    split_multi_waits(nc, max_waits=1)
    return nc


# =============================== host side ==================================

def prep_core_inputs(inputs, b, half):
    import ml_dtypes
    f32 = np.float32
    bf16 = ml_dtypes.bfloat16
    d0 = half * HH

    def bf(a):
        return np.ascontiguousarray(np.asarray(a, f32).astype(bf16))

    def fc(a):
        return np.ascontiguousarray(np.asarray(a, f32))

    bnscale = inputs['bn_g'] / np.sqrt(np.float32(1.0 + 1e-5))
    w1 = inputs['conv1_w'][:, :, 0, 0] * bnscale[:, None]
    b1 = inputs['conv1_b'] * bnscale + inputs['bn_b']

    def diag9(w, nch):
        # (nch,1,3,3) -> (nch, 9*nch): column block per tap = diag(w[:,dy,dx])
        out = np.zeros((nch, 9 * nch), f32)
        for tap in range(9):
            dy, dx = tap // 3, tap % 3
            blk = out[:, tap * nch:(tap + 1) * nch]
            np.fill_diagonal(blk, w[:, 0, dy, dx])
        return out

    inw = inputs['ss_in_w']
    inw_packed = np.concatenate([inw[:DI], inw[DI + d0:DI + d0 + HH]], 0)

    sscd = diag9(inputs['ss_conv_w'], DI)        # (192, 9*192)
    sc0 = np.zeros((128, 9 * 128), f32)
    sc1 = np.zeros((64, 9 * 64), f32)
    for tap in range(9):
        blk = sscd[:, tap * DI:(tap + 1) * DI]
        sc0[:, tap * 128:(tap + 1) * 128] = blk[0:128, 0:128]
        sc1[:, tap * 64:(tap + 1) * 64] = blk[128:192, 128:192]

    sel = np.zeros((DI, HH), f32)
    sel[np.arange(d0, d0 + HH), np.arange(HH)] = 1.0

    xp = inputs['ss_xproj_w']
    xpTa = np.zeros((128, K4 * 96), f32)
    xpTb = np.zeros((64, K4 * 96), f32)
    for k in range(K4):
        xpT = np.asarray(xp[k]).T
        for dst, (a0, a1) in [(0, (0, R)), (32, (R, R + N)), (64, (R + N, R + 2 * N))]:
            xpTa[:, k * 96 + dst:k * 96 + dst + a1 - a0] = xpT[0:128, a0:a1]
            xpTb[:, k * 96 + dst:k * 96 + dst + a1 - a0] = xpT[128:192, a0:a1]

    dtw = inputs['ss_dt_w']
    dtwT = np.zeros((R, K4 * HH), f32)
    for k in range(K4):
        dtwT[:, k * HH:(k + 1) * HH] = np.asarray(dtw[k])[d0:d0 + HH, :].T

    dtb_full = np.asarray(inputs['ss_dt_b'])
    Alog = np.asarray(inputs['ss_Alog']).reshape(K4, DI, N)
    Dv = np.asarray(inputs['ss_D']).reshape(K4, DI)
    dtb_p = np.zeros((128, NT), f32)
    Ap = np.zeros((128, NT * N), f32)
    Dp = np.zeros((128, NT), f32)
    for (j, o0, o1, k, dd0, dd1) in SECTIONS:
        dtb_p[o0:o1, j] = dtb_full[k, d0 + dd0:d0 + dd1]
        Ap[o0:o1, j * N:(j + 1) * N] = -np.exp(Alog[k, d0 + dd0:d0 + dd1])
        Dp[o0:o1, j] = Dv[k, d0 + dd0:d0 + dd1]

    gw = np.asarray(inputs['gwc_w'])
    gT = np.zeros((24, 9 * CH), f32)
    for tap in range(9):
        dy, dx = tap // 3, tap % 3
        blk = np.zeros((24, CH), f32)
        blk[0:12, 0:48] = gw[0:48, :, dy, dx].T
        blk[12:24, 48:96] = gw[48:96, :, dy, dx].T
        gT[:, tap * CH:(tap + 1) * CH] = blk

    m = {
        'x': fc(np.asarray(inputs['x'])[b].reshape(CIN, L)),
        'w1T': fc(w1.T), 'b1': fc(b1[:, None]),
        'linT': fc(np.asarray(inputs['lin_w']).T),
        'linb': fc(np.asarray(inputs['lin_b'])[:, None]),
        'dw1dg': diag9(inputs['dw1_w'], CH),
        'dw1b': fc(np.asarray(inputs['dw1_b'])[:, None]),
        'dw2dg': diag9(inputs['dw2_w'], CH),
        'dw2b': fc(np.asarray(inputs['dw2_b'])[:, None]),
        'inwT': bf(inw_packed.T),
        'sc0dg': bf(sc0), 'sc1dg': bf(sc1),
        'scb0': fc(np.asarray(inputs['ss_conv_b'])[0:128, None]),
        'scb1': fc(np.asarray(inputs['ss_conv_b'])[128:192, None]),
        'sel0': bf(sel[0:128]), 'sel1': bf(sel[128:192]),
        'xpTa': bf(xpTa), 'xpTb': bf(xpTb),
        'dtwT': bf(dtwT), 'dtb': dtb_p, 'Ap': Ap, 'Dp': Dp,
        'ident': bf(np.eye(128, dtype=f32)),
        'outng': fc(np.asarray(inputs['ss_outn_g'])[d0:d0 + HH, None]),
        'outnb': fc(np.asarray(inputs['ss_outn_b'])[d0:d0 + HH, None]),
        'outwT': bf(np.asarray(inputs['ss_out_w'])[:, d0:d0 + HH].T),
        'ag1T': bf(np.asarray(inputs['ag1_w'])[:, :, 0, 0].T),
        'ag1b': fc(np.asarray(inputs['ag1_b'])[:, None]),
        'ag2T': bf(np.asarray(inputs['ag2_w'])[:, :, 0, 0].T),
        'ag2b': fc(np.asarray(inputs['ag2_b'])[:, None]),
        'lng': fc(np.asarray(inputs['ln_g'])[:, None]),
        'lnb': fc(np.asarray(inputs['ln_b'])[:, None]),
        'sq1T': bf(np.asarray(inputs['sq1_w'])[:, :, 0, 0].T),
        'sq2T': bf(np.asarray(inputs['sq2_w'])[:, :, 0, 0].T),
        'gwcT': bf(gT),
        'gwcb': fc(np.asarray(inputs['gwc_b'])[:, None]),
        'pw1T': bf(np.asarray(inputs['pwc1_w'])[:, :, 0, 0].T),
        'pw2T': bf(np.asarray(inputs['pwc2_w'])[:, :, 0, 0].T),
        'finT': bf(np.asarray(inputs['fin_w']).T),
        'finb': fc(np.asarray(inputs['fin_b'])[:, None]),
    }
    return m


_NC_CACHE = {}


def get_nc():
    if 'nc' not in _NC_CACHE:
        _NC_CACHE['nc'] = build_nc()
    return _NC_CACHE['nc']


def kernel(**inputs):
    from concourse.bass_utils import run_bass_kernel_spmd
    nc = get_nc()
    in_maps = [prep_core_inputs(inputs, c // 2, c % 2) for c in range(8)]
    res = run_bass_kernel_spmd(nc, in_maps, core_ids=list(range(8)))
    out = np.zeros((B_, COUT, H, W), np.float32)
    for b in range(B_):
        out[b] = res.results[2 * b]['out'].reshape(COUT, H, W)
    return out

